# revision 15
# baseline (speedup 1.0000x reference)
"""Trainium2 Bass kernel for nn_CalibrationNetwork (dense_mlp).

Network (per sample b with judge j = judge_ids[b], per question q):
    z1 = sigmoid([1,x] @ (W1+W1_a[j])[q])        # [6]->[128]
    z2 = sigmoid([1,z1] @ (W2+W2_a[j]))          # [129]->[128]
    out = softmax([1,z2] @ (V+V_a[j])[q])        # [129]->[5]

Strategy (v2):
  - Data parallel over 8 cores; per-judge weights replicated.
  - Host computes the tiny L1 (4% of FLOPs) exactly in f32 and ships
    z1 = sigmoid(..) as bf16 [H1, q-major samples]; host also applies the
    output bias + softmax. The device runs only the two big stages:
      L2: m = z1 @ 0.5*W2c[1:]  (psum) ; t2 = tanh(m + 0.5*W2c[0])  (ACT)
      L3: logits^T[35, n] = sum_q t2_q @ Vpad[j,q]  (psum-accumulated
          matmuls with zero-padded V stationaries -> no z2 transpose)
    so sigmoid(s)=0.5+0.5*tanh(s/2) folding keeps a single ACT table set.
  - Host groups samples by judge with identical per-judge capacities on
    every core (one static SPMD program). Segments are software-pipelined
    2 deep: tensor stream is L2_i, L3_{i-1} so the tanh of segment i runs
    under the next segment's matmuls; psum = 4 banks/segment x 2 bufs.
  - L3 psum [35, C] lives in the spare slot of the last L2 psum bank; DVE
    copies it to a logits SBUF tile; output is one clean [35, ncap] f32
    stream DMA'd in a few group-sized chunks.
"""

import sys

import numpy as np

if "/opt/trn_rl_repo" not in sys.path:
    sys.path.insert(0, "/opt/trn_rl_repo")

B, J, Q, O, H1, H2 = 16384, 12, 7, 5, 128, 128
QO = Q * O  # 35
NCORES = 8
CMAX = 256  # max samples per device segment (psum: 2*CMAX <= 512)
GS = 3  # segments per output-DMA group
ZPBUFS = 3  # t2 tile buffering depth


def _bf16():
    import ml_dtypes

    return ml_dtypes.bfloat16


def _plan(judge_ids):
    """Distribute samples: per judge j, split its samples evenly over the 8
    cores and pad each core's share to a common capacity C_j, so every core
    sees identical segment geometry (one compiled program, SPMD)."""
    jid = np.asarray(judge_ids).astype(np.int64)
    order = np.argsort(jid, kind="stable")
    sorted_j = jid[order]
    caps = []
    parts = []  # parts[j][c] = per-core padded index array (len caps[j])
    for j in range(J):
        lo = np.searchsorted(sorted_j, j, side="left")
        hi = np.searchsorted(sorted_j, j, side="right")
        idx_j = order[lo:hi]
        cnt = hi - lo
        if cnt == 0:
            caps.append(0)
            parts.append(None)
            continue
        cj = -(-cnt // NCORES)  # ceil
        cj = (cj + 3) // 4 * 4  # 4-elem multiple keeps bf16 slices 8B-aligned
        caps.append(cj)
        pj = []
        for c in range(NCORES):
            part = idx_j[c::NCORES]
            if len(part) < cj:
                pad_val = part[-1] if len(part) else idx_j[0]
                part = np.concatenate(
                    [part, np.full(cj - len(part), pad_val, dtype=part.dtype)]
                )
            pj.append(part)
        parts.append(pj)
    core_idx = [
        np.concatenate([parts[j][c] for j in range(J) if parts[j] is not None])
        for c in range(NCORES)
    ]
    ncap = int(sum(caps))
    # segments (judge, start, size) with size <= CMAX; identical on all cores
    segs = []
    n0 = 0
    for j in range(J):
        c = caps[j]
        while c > 0:
            s = min(c, CMAX)
            segs.append((j, n0, s))
            n0 += s
            c -= s
    assert n0 == ncap
    return core_idx, parts, caps, segs, ncap


def _fold_weights(W1, W1_a, W2, W2_a, V, V_a):
    """Per-judge weight transforms (all tiny)."""
    f32 = np.float32
    bf16 = _bf16()
    W1c = (W1[None] + W1_a).astype(f32)  # [J,Q,6,H1] (host L1, exact)
    W2c = (W2[None] + W2_a).astype(f32)  # [J,129,H2]
    w2s = np.ascontiguousarray(
        (0.5 * W2c[:, 1:, :]).transpose(1, 0, 2).reshape(H1, J * H2)
    ).astype(bf16)
    b2s = np.ascontiguousarray(0.5 * W2c[:, 0, :].T).astype(f32)  # [H2,J]
    Vc = (V[None] + V_a).astype(f32)  # [J,Q,129,O]
    Vm = 0.5 * Vc[:, :, 1:, :]  # [J,Q,H2,O]
    # zero-padded per-(j,q) stationaries: matmul out base partition must be
    # 0/32/64, so each q's [H2,5] block sits in its own column range and the
    # 7 matmuls accumulate into one [35, C] psum block.
    vsp = np.zeros((J, Q, H2, QO), f32)
    for q in range(Q):
        vsp[:, q, :, q * O : (q + 1) * O] = Vm[:, q]
    vs = np.ascontiguousarray(vsp.transpose(2, 0, 1, 3).reshape(H2, J * Q * QO)).astype(bf16)
    bV = (Vc[:, :, 0, :] + 0.5 * Vc[:, :, 1:, :].sum(2)).astype(f32)  # [J,Q,O]
    return W1c, w2s, b2s, vs, bV


def _host_l1(x, parts, caps, segs, ncap, W1c):
    """z1 = sigmoid([1,x] @ W1c[j,q]) on the host, laid out per core as
    [H1, judge-major (q, n)] bf16 ready to be the L2 matmul rhs."""
    bf16 = _bf16()
    xb = np.empty((x.shape[0], Q, O + 1), np.float32)
    xb[:, :, 0] = 1.0
    xb[:, :, 1:] = x
    z1 = [np.empty((H1, Q * ncap), bf16) for _ in range(NCORES)]
    off = 0
    for j in range(J):
        C = caps[j]
        if C == 0:
            continue
        idx = np.concatenate([parts[j][c] for c in range(NCORES)])  # [8C]
        s = np.matmul(xb[idx].transpose(1, 0, 2), W1c[j])  # [Q, 8C, H1]
        zj = (1.0 / (1.0 + np.exp(-s))).astype(bf16)
        for c in range(NCORES):
            blk = zj[:, c * C : (c + 1) * C, :]  # [Q, C, H1]
            z1[c][:, Q * off : Q * (off + C)] = np.ascontiguousarray(
                blk.transpose(2, 0, 1)
            ).reshape(H1, Q * C)
        off += C
    assert off == ncap
    return z1


def _seg_geom(C):
    spb = min(512 // C, 4)  # q-slots per psum bank
    nbanks = -(-Q // spb)
    lb = (Q - 1) // spb  # last bank used by L2
    c3 = ((Q - 1) % spb) * C + C  # L3 psum col offset within that bank
    assert nbanks <= 4 and c3 + C <= 512
    return spb, nbanks, lb, c3


def _build_program(ncap, segs, reps=1):
    import contextlib

    import concourse.bass as bass  # noqa: F401
    import concourse.tile as tile
    from concourse import bacc, mybir

    f32 = mybir.dt.float32
    bf16 = mybir.dt.bfloat16
    AF = mybir.ActivationFunctionType

    nc = bacc.Bacc("TRN2", target_bir_lowering=False, debug=False, num_devices=NCORES)
    d_z1 = nc.dram_tensor("z1", [H1, Q * ncap], bf16, kind="ExternalInput")
    d_w2 = nc.dram_tensor("w2s", [H1, J * H2], bf16, kind="ExternalInput")
    d_b2 = nc.dram_tensor("b2s", [H2, J], f32, kind="ExternalInput")
    d_vs = nc.dram_tensor("vs", [H2, J * Q * QO], bf16, kind="ExternalInput")
    d_out = nc.dram_tensor("out", [QO, ncap], f32, kind="ExternalOutput")

    with tile.TileContext(nc) as tc:
        with (
            tc.tile_pool(name="singles", bufs=1) as singles,
            tc.tile_pool(name="zp", bufs=ZPBUFS) as zp,
            tc.tile_pool(name="pp", bufs=2, space="PSUM") as pp,
        ):
            sw2 = singles.tile([H1, J * H2], bf16)
            sz1 = singles.tile([H1, Q * ncap], bf16)
            sb2 = singles.tile([H2, J], f32)
            svs = singles.tile([H2, J * Q * QO], bf16)
            slog = singles.tile([QO, ncap], f32)
            scratch = singles.tile([1, 8], f32)

            # Preload the ACT table set (tanh) during the DMA fill so the
            # ~1.3us ACT_TABLE_LOAD is off the first tanh's critical path.
            nc.vector.memset(scratch[:], 0.0)
            nc.scalar.activation(out=scratch[:], in_=scratch[:], func=AF.Tanh)

            # DMA issue order = first-use order, split across two issuing
            # engines so issue cost (~0.7us each) overlaps: Sync streams the
            # z1 slabs, GpSimd covers the small weight tensors. z1 goes in
            # graded slabs (1,2,3,.. segments) so segment 0 can start ASAP
            # while later transfers batch up for DMA efficiency.
            nc.sync.dma_start(out=sw2[:], in_=d_w2.ap())
            nc.gpsimd.dma_start(out=sb2[:], in_=d_b2.ap())
            nc.gpsimd.dma_start(out=svs[:], in_=d_vs.ap())
            slabs = []
            k = 0
            w = 1
            while k < len(segs):
                slabs.append(segs[k : k + w])
                k += w
                w = min(w + 1, 3)
            for slab in slabs:
                a = Q * slab[0][1]
                b = Q * (slab[-1][1] + slab[-1][2])
                nc.sync.dma_start(out=sz1[:, a:b], in_=d_z1.ap()[:, a:b])

            def emit_l2(j, n0, C):
                spb, nbanks, lb, c3 = _seg_geom(C)
                p = pp.tile([128, 4, 512], f32, tag="ps")
                for b in range(nbanks):
                    nq = min(spb, Q - b * spb)
                    w = nq * C
                    nc.tensor.matmul(
                        out=p[:, b, 0:w],
                        lhsT=sw2[:, j * H2 : (j + 1) * H2],
                        rhs=sz1[:, Q * n0 + b * spb * C : Q * n0 + b * spb * C + w],
                        start=True,
                        stop=True,
                    )
                t2 = zp.tile([128, 8 * CMAX], bf16, tag="t2")
                nc.scalar.activation(
                    out=t2[:, : nbanks * spb * C].rearrange("p (b s) -> p b s", b=nbanks),
                    in_=p[:, :nbanks, : spb * C],
                    func=AF.Tanh,
                    bias=sb2[:, j : j + 1],
                )
                return p, t2

            def emit_l3(j, n0, C, p, t2):
                spb, nbanks, lb, c3 = _seg_geom(C)
                for q in range(Q):
                    nc.tensor.matmul(
                        out=p[0:QO, lb, c3 : c3 + C],
                        lhsT=svs[:, (j * Q + q) * QO : (j * Q + q + 1) * QO],
                        rhs=t2[:, q * C : (q + 1) * C],
                        start=(q == 0),
                        stop=(q == Q - 1),
                    )
                nc.vector.tensor_copy(
                    out=slog[:, n0 : n0 + C], in_=p[0:QO, lb, c3 : c3 + C]
                )

            loop_cm = tc.For_i(0, reps, 1) if reps > 1 else contextlib.nullcontext()
            with loop_cm:
                prev = None
                done = []  # (n0, C) of segments whose logits are in slog
                g0 = 0  # start col of the pending output group
                for i, (j, n0, C) in enumerate(segs):
                    cur = (j, n0, C) + emit_l2(j, n0, C)
                    if prev is not None:
                        emit_l3(*prev)
                        done.append((prev[1], prev[2]))
                        if len(done) >= GS:
                            gend = done[-1][0] + done[-1][1]
                            nc.gpsimd.dma_start(
                                out=d_out.ap()[:, g0:gend], in_=slog[:, g0:gend]
                            )
                            g0 = gend
                            done = []
                    prev = cur
                emit_l3(*prev)
                gend = prev[1] + prev[2]
                nc.gpsimd.dma_start(out=d_out.ap()[:, g0:gend], in_=slog[:, g0:gend])

    nc.compile()
    return nc


def _prepare(x, judge_ids, W1, W1_a, W2, W2_a, V, V_a):
    f32 = np.float32
    x = np.ascontiguousarray(np.asarray(x), dtype=f32)
    jid = np.asarray(judge_ids)
    W1c, w2s, b2s, vs, bV = _fold_weights(
        np.asarray(W1, f32),
        np.asarray(W1_a, f32),
        np.asarray(W2, f32),
        np.asarray(W2_a, f32),
        np.asarray(V, f32),
        np.asarray(V_a, f32),
    )
    core_idx, parts, caps, segs, ncap = _plan(jid)
    z1 = _host_l1(x, parts, caps, segs, ncap, W1c)
    in_maps = [
        {"z1": z1[c], "w2s": w2s, "b2s": b2s, "vs": vs} for c in range(NCORES)
    ]

    def post(outs):
        """outs[c] = device logits^T [35, ncap] (no bias). Host adds the
        bias table and softmaxes."""
        out_full = np.empty((x.shape[0], Q, O), f32)
        for c in range(NCORES):
            lg = np.asarray(outs[c], f32).T.reshape(ncap, Q, O).copy()
            lg += bV[jid[core_idx[c]].astype(np.int64)]
            lg -= lg.max(-1, keepdims=True)
            np.exp(lg, out=lg)
            lg /= lg.sum(-1, keepdims=True)
            out_full[core_idx[c]] = lg
        return out_full

    return core_idx, segs, ncap, in_maps, post


def kernel(x, judge_ids, W1, W1_a, W2, W2_a, V, V_a):
    from concourse import bass_utils

    core_idx, segs, ncap, in_maps, post = _prepare(
        x, judge_ids, W1, W1_a, W2, W2_a, V, V_a
    )
    nc = _build_program(ncap, segs)
    res = bass_utils.run_bass_kernel_spmd(nc, in_maps, core_ids=list(range(NCORES)))
    return post([res.results[c]["out"] for c in range(NCORES)])
